# revision 5
# baseline (speedup 1.0000x reference)
"""Multi-head causal attention (B=2, T=2048, C=2048, H=16, D=128) on 8 TRN2
NeuronCores.

Sharding: core = (batch, head_group): cores 0-3 -> batch 0, cores 4-7 ->
batch 1; head_group hg = core % 4 owns heads [4*hg, 4*hg+4).  Each core:
  qkv projection for its 4 heads (q/k transposed layout, v natural layout),
  multiplicative RoPE, causal attention with transposed scores, and the
  out-projection partial product over its 512 feature columns.
Host: per-batch sum of the 4 partial outputs + out bias (the "all-reduce").
"""

import math
from contextlib import ExitStack

import ml_dtypes
import numpy as np

import concourse.bass as bass
import concourse.tile as tile
from concourse import bacc, mybir

BF16 = mybir.dt.bfloat16
FP32 = mybir.dt.float32
AF = mybir.ActivationFunctionType

B, T, C, H, D = 2, 2048, 2048, 16, 128
HPC = 4          # heads per core
P = 128
KO = C // P      # 16 k-tiles for the projection contraction
MCH = T // 512   # 4 m-chunks of 512 tokens
NJB = T // P     # 16 key blocks
NCI = T // 512   # 4 query chunks


def build_bass(iters: int = 1):
    nc = bacc.Bacc("TRN2", target_bir_lowering=False, debug=False, num_devices=8)

    xT_d = nc.dram_tensor("xT", [C, T], BF16, kind="ExternalInput").ap()
    wT_d = nc.dram_tensor("wT", [C, 3 * HPC * D], BF16, kind="ExternalInput").ap()
    bqk_d = nc.dram_tensor("bqk", [P, 2 * HPC], FP32, kind="ExternalInput").ap()
    bv_d = nc.dram_tensor("bv", [P, HPC * D], FP32, kind="ExternalInput").ap()
    embq_d = nc.dram_tensor("embq", [P, T], BF16, kind="ExternalInput").ap()
    embk_d = nc.dram_tensor("embk", [P, T], BF16, kind="ExternalInput").ap()
    woT_d = nc.dram_tensor("woT", [HPC * D, C], BF16, kind="ExternalInput").ap()
    y_d = nc.dram_tensor("y", [T, C], FP32, kind="ExternalOutput").ap()

    with tile.TileContext(nc) as tc, ExitStack() as ctx:
        cpool = ctx.enter_context(tc.tile_pool(name="consts", bufs=1))
        qk_pool = ctx.enter_context(tc.tile_pool(name="qk", bufs=1))
        v_pool = ctx.enter_context(tc.tile_pool(name="v", bufs=1))
        a_pool = ctx.enter_context(tc.tile_pool(name="aT", bufs=1))
        x_pool = ctx.enter_context(tc.tile_pool(name="xt", bufs=2))
        s_pool = ctx.enter_context(tc.tile_pool(name="es", bufs=4))
        r_pool = ctx.enter_context(tc.tile_pool(name="recip", bufs=2))
        rb_pool = ctx.enter_context(tc.tile_pool(name="recipb", bufs=2))
        y_pool = ctx.enter_context(tc.tile_pool(name="ys", bufs=3))
        ps_pool = ctx.enter_context(tc.tile_pool(name="ps", bufs=8, space="PSUM"))

        # ---- constants, loaded once ----
        w_sb = cpool.tile([P, KO, 3 * HPC * D], BF16)   # qkv weightsT
        nc.sync.dma_start(w_sb[:], wT_d.rearrange("(ko p) n -> p ko n", p=P))
        woT_sb = cpool.tile([P, HPC, C], BF16)
        nc.sync.dma_start(woT_sb[:], woT_d.rearrange("(hh p) n -> p hh n", p=P))
        embq_sb = cpool.tile([P, T], BF16)
        nc.sync.dma_start(embq_sb[:], embq_d)
        embk_sb = cpool.tile([P, T], BF16)
        nc.sync.dma_start(embk_sb[:], embk_d)
        bqk_sb = cpool.tile([P, 2 * HPC], FP32)
        nc.sync.dma_start(bqk_sb[:], bqk_d)
        bv_sb = cpool.tile([P, HPC * D], FP32)
        nc.sync.dma_start(bv_sb[:], bv_d)
        ones_sb = cpool.tile([P, 1], BF16)
        nc.vector.memset(ones_sb[:], 1.0)

        def body():
            # persistent activations for one full iteration
            qkT = qk_pool.tile([P, 2 * HPC, T], BF16)    # q/k featT x tokens
            v_sb = v_pool.tile([P, NJB, HPC * D], BF16)  # tokens x v feats
            aT = a_pool.tile([P, HPC, T], BF16)          # attn outT per head

            # ---- phase 1: qkv projection ----
            xTr = xT_d.rearrange("(ko p) m -> p ko m", p=P)
            for mc in range(MCH):
                ms = slice(mc * 512, (mc + 1) * 512)
                xt = x_pool.tile([P, KO, 512], BF16)
                nc.sync.dma_start(xt[:], xTr[:, :, ms])
                # q/k blocks: psum [feat 128, tok 512]
                ps_qk = [ps_pool.tile([P, 512], FP32, tag="mm", name=f"psqk{mc}_{f}") for f in range(8)]
                for ko in range(KO):
                    for f in range(8):
                        nc.tensor.matmul(
                            ps_qk[f][:],
                            lhsT=w_sb[:, ko, f * P:(f + 1) * P],
                            rhs=xt[:, ko, :],
                            start=(ko == 0),
                            stop=(ko == KO - 1),
                        )
                for f in range(8):
                    nc.scalar.activation(
                        out=qkT[:, f, ms], in_=ps_qk[f][:], func=AF.Identity,
                        bias=bqk_sb[:, f:f + 1],
                    )
                    # multiplicative RoPE (scale 1/sqrt(D) folded into embq)
                    emb = embq_sb if f < HPC else embk_sb
                    nc.vector.tensor_mul(
                        out=qkT[:, f, ms], in0=qkT[:, f, ms], in1=emb[:, ms]
                    )
                # v blocks: psum [tok 128, feat 512]
                ps_v = [ps_pool.tile([P, 512], FP32, tag="mm", name=f"psv{mc}_{s}") for s in range(4)]
                for ko in range(KO):
                    for s in range(4):
                        nc.tensor.matmul(
                            ps_v[s][:],
                            lhsT=xt[:, ko, s * P:(s + 1) * P],
                            rhs=w_sb[:, ko, 2 * HPC * D:3 * HPC * D],
                            start=(ko == 0),
                            stop=(ko == KO - 1),
                        )
                for s in range(4):
                    nc.vector.tensor_add(
                        out=v_sb[:, mc * 4 + s, :], in0=ps_v[s][:], in1=bv_sb[:]
                    )

            # ---- phase 2: causal attention, scores transposed [j, i] ----
            for h in range(HPC):
                for ci in range(NCI):
                    iss = slice(ci * 512, (ci + 1) * 512)
                    njb = 4 * (ci + 1)
                    ps_o = ps_pool.tile([P, 512], FP32, tag="mm", name=f"pso{h}_{ci}")
                    ps_d = ps_pool.tile([P, 512], FP32, tag="mm", name=f"psd{h}_{ci}")
                    for jb in range(njb):
                        ps_s = ps_pool.tile([P, 512], FP32, tag="mm", name=f"pss{h}_{ci}_{jb}")
                        nc.tensor.matmul(
                            ps_s[:],
                            lhsT=qkT[:, HPC + h, jb * P:(jb + 1) * P],
                            rhs=qkT[:, h, iss],
                            start=True, stop=True,
                        )
                        es = s_pool.tile([P, 512], BF16)
                        nc.scalar.activation(out=es[:], in_=ps_s[:], func=AF.Exp)
                        if jb >= 4 * ci:  # diagonal block: zero where j > i
                            delta = jb * P - ci * 512
                            nc.gpsimd.affine_select(
                                out=es[:], in_=es[:],
                                compare_op=mybir.AluOpType.is_ge,
                                fill=0.0, base=-delta,
                                pattern=[[1, 512]], channel_multiplier=-1,
                            )
                        nc.tensor.matmul(
                            ps_o[:],
                            lhsT=v_sb[:, jb, h * D:(h + 1) * D],
                            rhs=es[:],
                            start=(jb == 0), stop=(jb == njb - 1),
                        )
                        nc.tensor.matmul(
                            ps_d[0:1, :],
                            lhsT=ones_sb[:],
                            rhs=es[:],
                            start=(jb == 0), stop=(jb == njb - 1),
                        )
                    rr = r_pool.tile([1, 512], FP32)
                    nc.vector.reciprocal(out=rr[:], in_=ps_d[0:1, :])
                    rb = rb_pool.tile([P, 512], FP32)
                    nc.gpsimd.partition_broadcast(rb[:], rr[:])
                    nc.vector.tensor_mul(out=aT[:, h, iss], in0=ps_o[:], in1=rb[:])

            # ---- phase 3: out projection partial ----
            for mb in range(T // P):
                for cc in range(C // 512):
                    ps_y = ps_pool.tile([P, 512], FP32, tag="mm", name=f"psy{mb}_{cc}")
                    for h in range(HPC):
                        nc.tensor.matmul(
                            ps_y[:],
                            lhsT=aT[:, h, mb * P:(mb + 1) * P],
                            rhs=woT_sb[:, h, cc * 512:(cc + 1) * 512],
                            start=(h == 0), stop=(h == HPC - 1),
                        )
                    ys = y_pool.tile([P, 512], FP32)
                    nc.vector.tensor_copy(out=ys[:], in_=ps_y[:])
                    nc.sync.dma_start(
                        y_d[mb * P:(mb + 1) * P, cc * 512:(cc + 1) * 512], ys[:]
                    )

        if iters == 1:
            body()
        else:
            with tc.For_i(0, iters, 1):
                body()

    nc.compile()
    return nc


def _rope_emb():
    freqs = np.arange(0, D, 2, dtype=np.float32) / D
    angles = np.arange(T, dtype=np.float32)[:, None] * freqs[None, :]
    return np.concatenate([np.sin(angles), np.cos(angles)], axis=-1)  # [T, D]


def make_in_maps(x, qkv_w, qkv_b, out_w):
    bf = ml_dtypes.bfloat16
    emb = _rope_emb()
    embq = np.ascontiguousarray((emb / math.sqrt(D)).T).astype(bf)  # [D, T]
    embk = np.ascontiguousarray(emb.T).astype(bf)
    in_maps = []
    for core in range(8):
        b, hg = divmod(core, 4)
        rq = slice(hg * 512, (hg + 1) * 512)
        rk = slice(C + hg * 512, C + (hg + 1) * 512)
        rv = slice(2 * C + hg * 512, 2 * C + (hg + 1) * 512)
        w_slice = np.concatenate([qkv_w[rq], qkv_w[rk], qkv_w[rv]], axis=0)
        wT = np.ascontiguousarray(w_slice.T).astype(bf)  # [C, 1536]
        bq = qkv_b[rq].reshape(HPC, P).T  # [128, 4] per-block columns
        bk = qkv_b[rk].reshape(HPC, P).T
        bqk = np.ascontiguousarray(np.concatenate([bq, bk], axis=1)).astype(np.float32)
        bv = np.ascontiguousarray(
            np.broadcast_to(qkv_b[rv][None, :], (P, HPC * D))
        ).astype(np.float32)
        woT = np.ascontiguousarray(out_w[:, hg * 512:(hg + 1) * 512].T).astype(bf)
        xT = np.ascontiguousarray(x[b].T).astype(bf)  # [C, T]
        in_maps.append({
            "xT": xT, "wT": wT, "bqk": bqk, "bv": bv,
            "embq": embq, "embk": embk, "woT": woT,
        })
    return in_maps


_NC_CACHE = {}


def kernel(x, qkv_w, qkv_b, out_w, out_b):
    from concourse.bass_utils import run_bass_kernel_spmd

    x = np.asarray(x, dtype=np.float32)
    qkv_w = np.asarray(qkv_w, dtype=np.float32)
    qkv_b = np.asarray(qkv_b, dtype=np.float32)
    out_w = np.asarray(out_w, dtype=np.float32)
    out_b = np.asarray(out_b, dtype=np.float32)

    if "nc" not in _NC_CACHE:
        _NC_CACHE["nc"] = build_bass()
    nc = _NC_CACHE["nc"]

    in_maps = make_in_maps(x, qkv_w, qkv_b, out_w)
    res = run_bass_kernel_spmd(nc, in_maps, core_ids=list(range(8)))
    out = np.empty((B, T, C), dtype=np.float32)
    for b in range(B):
        acc = res.results[4 * b]["y"].astype(np.float32)
        for hg in range(1, 4):
            acc = acc + res.results[4 * b + hg]["y"]
        out[b] = acc + out_b[None, :]
    return out


# revision 31
# speedup vs baseline: 4.4826x; 4.4826x over previous
"""Multi-head causal attention (B=2, T=2048, C=2048, H=16, D=128) on 8 TRN2
NeuronCores.

Sharding: core = (batch, head_group): cores 0-3 -> batch 0, cores 4-7 ->
batch 1; head_group hg = core % 4 owns heads [4*hg, 4*hg+4).  Each core:
  qkv projection for its 4 heads (q/k transposed layout, v natural layout),
  multiplicative RoPE, causal attention with transposed scores, and the
  out-projection partial product over its 512 feature columns.
Host: per-batch sum of the 4 partial outputs + out bias (the "all-reduce").

Emission interleaves attention/out-proj chunks with projection m-chunks
(causality: query chunk ci only needs tokens < 512*(ci+1)) so the scalar/
vector-heavy softmax work overlaps the PE-heavy projection.
"""

import math
from contextlib import ExitStack

import ml_dtypes
import numpy as np

import concourse.bass as bass
import concourse.tile as tile
from concourse import bacc, bass_isa, mybir

BF16 = mybir.dt.bfloat16
FP32 = mybir.dt.float32
AF = mybir.ActivationFunctionType

B, T, C, H, D = 2, 2048, 2048, 16, 128
HPC = 4          # heads per core
P = 128
KO = C // P      # 16 k-tiles for the projection contraction
MCH = T // 512   # 4 m-chunks of 512 tokens
NJB = T // P     # 16 key blocks
NCI = T // 512   # 4 query chunks


def build_bass(iters: int = 1, timing_io: bool = False, no_pool: bool = False,
               only: str = ""):
    nc = bacc.Bacc("TRN2", target_bir_lowering=False, debug=False, num_devices=8)

    # timing_io: big tensors become Internal DRAM (contents garbage) so each
    # run ships only a tiny input/output over the axon tunnel
    kind = "Internal" if timing_io else "ExternalInput"

    def in_tensor(name, shape, dtype):
        if timing_io:
            return nc.dram_tensor(name, shape, dtype).ap()
        return nc.dram_tensor(name, shape, dtype, kind="ExternalInput").ap()

    xT_d = in_tensor("xT", [C, T], BF16)
    wT_d = in_tensor("wT", [C, 3 * HPC * D], BF16)
    bqk_d = nc.dram_tensor("bqk", [P, 2 * HPC], FP32, kind="ExternalInput").ap()
    bv_d = in_tensor("bv", [P, HPC * D], FP32)
    embq_d = in_tensor("embq", [P, T], BF16)
    embk_d = in_tensor("embk", [P, T], BF16)
    woT_d = in_tensor("woT", [HPC * D, C], BF16)
    dmask_d = in_tensor("dmask", [P, 4 * 512], BF16)
    if timing_io:
        y_d = nc.dram_tensor("y", [T, C], FP32).ap()
        yext_d = nc.dram_tensor("yext", [P, 512], FP32, kind="ExternalOutput").ap()
    else:
        y_d = nc.dram_tensor("y", [T, C], FP32, kind="ExternalOutput").ap()
        yext_d = None

    with tile.TileContext(nc) as tc, ExitStack() as ctx:
        cpool = ctx.enter_context(tc.tile_pool(name="consts", bufs=1))
        qk_pool = ctx.enter_context(tc.tile_pool(name="qk", bufs=1))
        v_pool = ctx.enter_context(tc.tile_pool(name="v", bufs=1))
        a_pool = ctx.enter_context(tc.tile_pool(name="aT", bufs=1))
        x_pool = ctx.enter_context(tc.tile_pool(name="xt", bufs=2))
        s_pool = ctx.enter_context(tc.tile_pool(name="es", bufs=6))
        r_pool = ctx.enter_context(tc.tile_pool(name="recip", bufs=2))
        rb_pool = ctx.enter_context(tc.tile_pool(name="recipb", bufs=2))
        y_pool = ctx.enter_context(tc.tile_pool(name="ys", bufs=4))
        ps_pool = ctx.enter_context(tc.tile_pool(name="ps", bufs=8, space="PSUM"))

        # const tiles (DMA emission deferred so xt chunk 0 leads the queue)
        w_sb = cpool.tile([P, KO, 3 * HPC * D], BF16)   # qkv weightsT
        woT_sb = cpool.tile([P, HPC, C], BF16)
        embq_sb = cpool.tile([P, T], BF16)
        embk_sb = cpool.tile([P, T], BF16)
        bqk_sb = cpool.tile([P, 2 * HPC], FP32)
        bv_sb = cpool.tile([P, HPC * D], FP32)
        ones_sb = cpool.tile([P, 1], BF16)
        ones_row = cpool.tile([1, P], FP32)
        dmask_sb = cpool.tile([P, 4 * 512], BF16)

        def load_consts():
            # smaller per-ko transfers so the first matmuls start early
            wTr = wT_d.rearrange("(ko p) n -> p ko n", p=P)
            for ko in range(KO):
                nc.sync.dma_start(w_sb[:, ko, :], wTr[:, ko, :])
            nc.sync.dma_start(bqk_sb[:], bqk_d)
            nc.sync.dma_start(bv_sb[:], bv_d)
            nc.sync.dma_start(embq_sb[:], embq_d)
            nc.sync.dma_start(embk_sb[:], embk_d)
            nc.sync.dma_start(woT_sb[:], woT_d.rearrange("(hh p) n -> p hh n", p=P))
            if no_pool:
                nc.sync.dma_start(dmask_sb[:], dmask_d)
            nc.vector.memset(ones_sb[:], 1.0)
            nc.vector.memset(ones_row[:], 1.0)

        xTr = xT_d.rearrange("(ko p) m -> p ko m", p=P)

        def proj_chunk(mc, qkT, v_sb, emit_consts):
            ms = slice(mc * 512, (mc + 1) * 512)
            xt = x_pool.tile([P, KO, 512], BF16, tag="xt", name=f"xt{mc}")
            nc.sync.dma_start(xt[:], xTr[:, :, ms])
            if emit_consts:
                load_consts()
            # three passes of 4 psum banks each: q, k, v
            for part in range(2):      # 0: q feats, 1: k feats
                ps = [ps_pool.tile([P, 512], FP32, tag="mm",
                                   name=f"psqk{mc}_{part}_{f}") for f in range(4)]
                for ko in range(KO):
                    for f in range(4):
                        fb = part * 4 + f
                        nc.tensor.matmul(
                            ps[f][:],
                            lhsT=w_sb[:, ko, fb * P:(fb + 1) * P],
                            rhs=xt[:, ko, :],
                            start=(ko == 0),
                            stop=(ko == KO - 1),
                        )
                for f in range(4):
                    fb = part * 4 + f
                    nc.scalar.activation(
                        out=qkT[:, fb, ms], in_=ps[f][:], func=AF.Identity,
                        bias=bqk_sb[:, fb:fb + 1],
                    )
                    # multiplicative RoPE (1/sqrt(D) folded into embq)
                    emb = embq_sb if part == 0 else embk_sb
                    nc.vector.tensor_mul(
                        out=qkT[:, fb, ms], in0=qkT[:, fb, ms], in1=emb[:, ms]
                    )
            ps_v = [ps_pool.tile([P, 512], FP32, tag="mm",
                                 name=f"psv{mc}_{s}") for s in range(4)]
            for ko in range(KO):
                for s in range(4):
                    nc.tensor.matmul(
                        ps_v[s][:],
                        lhsT=xt[:, ko, s * P:(s + 1) * P],
                        rhs=w_sb[:, ko, 2 * HPC * D:3 * HPC * D],
                        start=(ko == 0),
                        stop=(ko == KO - 1),
                    )
            for s in range(4):
                nc.vector.tensor_add(
                    out=v_sb[:, mc * 4 + s, :], in0=ps_v[s][:], in1=bv_sb[:]
                )

        def attn_chunk(h, ci, qkT, v_sb, aT):
            iss = slice(ci * 512, (ci + 1) * 512)
            njb = 4 * (ci + 1)
            ps_o = ps_pool.tile([P, 512], FP32, tag="mm", name=f"pso{h}_{ci}")
            ps_d = ps_pool.tile([P, 512], FP32, tag="mm", name=f"psd{h}_{ci}")
            acc = r_pool.tile([P, 512], FP32, tag="acc", name=f"acc{h}_{ci}")
            for jb in range(njb):
                ps_s = ps_pool.tile([P, 512], FP32, tag="mm",
                                    name=f"pss{h}_{ci}_{jb}")
                nc.tensor.matmul(
                    ps_s[:],
                    lhsT=qkT[:, HPC + h, jb * P:(jb + 1) * P],
                    rhs=qkT[:, h, iss],
                    start=True, stop=True,
                )
                es = s_pool.tile([P, 512], BF16, tag="es", name=f"es{h}_{ci}_{jb}")
                diag = jb >= 4 * ci
                delta = jb * P - ci * 512
                if no_pool:
                    nc.scalar.activation(out=es[:], in_=ps_s[:], func=AF.Exp)
                    if diag:  # multiplicative causal mask, const tile on DVE
                        di = delta // P
                        nc.vector.tensor_mul(
                            out=es[:], in0=es[:],
                            in1=dmask_sb[:, di * 512:(di + 1) * 512],
                        )
                elif diag:
                    # columns < delta are fully masked: skip their exp
                    if delta > 0:
                        nc.gpsimd.memset(es[:, :delta], 0.0)
                    nc.scalar.activation(
                        out=es[:, delta:], in_=ps_s[:, delta:], func=AF.Exp
                    )
                    nc.gpsimd.affine_select(
                        out=es[:, delta:], in_=es[:, delta:],
                        compare_op=mybir.AluOpType.is_ge,
                        fill=0.0, base=0,
                        pattern=[[1, 512 - delta]], channel_multiplier=-1,
                    )
                else:
                    nc.scalar.activation(out=es[:], in_=ps_s[:], func=AF.Exp)
                nc.tensor.matmul(
                    ps_o[:],
                    lhsT=v_sb[:, jb, h * D:(h + 1) * D],
                    rhs=es[:],
                    start=(jb == 0), stop=(jb == njb - 1),
                )
                if no_pool:
                    # denominator entirely on PE
                    nc.tensor.matmul(
                        ps_d[0:1, :],
                        lhsT=ones_sb[:],
                        rhs=es[:],
                        start=(jb == 0), stop=(jb == njb - 1),
                    )
                elif jb == 0:
                    nc.vector.tensor_copy(out=acc[:], in_=es[:])
                elif jb % 2 == 0:
                    nc.vector.tensor_add(out=acc[:], in0=acc[:], in1=es[:])
                else:
                    nc.tensor.matmul(
                        ps_d[0:1, :],
                        lhsT=ones_sb[:],
                        rhs=es[:],
                        start=(jb == 1), stop=(jb == njb - 1),
                    )
            rb = rb_pool.tile([P, 512], FP32, tag="rb", name=f"rb{h}_{ci}")
            if no_pool:
                rr = r_pool.tile([1, 512], FP32, tag="acc", name=f"rr{h}_{ci}")
                nc.vector.reciprocal(out=rr[:], in_=ps_d[0:1, :])
                # broadcast across partitions as a rank-1 outer product on PE
                ps_rb = ps_pool.tile([P, 512], FP32, tag="mm", name=f"psrb{h}_{ci}")
                nc.tensor.matmul(
                    ps_rb[:], lhsT=ones_row[:], rhs=rr[:], start=True, stop=True
                )
                nc.vector.tensor_copy(out=rb[:], in_=ps_rb[:])
            else:
                nc.vector.tensor_add(
                    out=acc[0:1, :], in0=acc[0:1, :], in1=ps_d[0:1, :]
                )
                nc.gpsimd.partition_all_reduce(
                    rb[:], acc[:], channels=P, reduce_op=bass_isa.ReduceOp.add
                )
                nc.vector.reciprocal(out=rb[:], in_=rb[:])
            nc.vector.tensor_mul(out=aT[:, h, iss], in0=ps_o[:], in1=rb[:])

        def outproj_chunk(mb, aT):
            for cc in range(C // 512):
                ps_y = ps_pool.tile([P, 512], FP32, tag="mm", name=f"psy{mb}_{cc}")
                for h in range(HPC):
                    nc.tensor.matmul(
                        ps_y[:],
                        lhsT=aT[:, h, mb * P:(mb + 1) * P],
                        rhs=woT_sb[:, h, cc * 512:(cc + 1) * 512],
                        start=(h == 0), stop=(h == HPC - 1),
                    )
                ys = y_pool.tile([P, 512], FP32, tag="ys", name=f"ys{mb}_{cc}")
                # alternate drain engine to split the load
                if (mb + cc) % 2 == 0:
                    nc.vector.tensor_copy(out=ys[:], in_=ps_y[:])
                else:
                    nc.scalar.copy(out=ys[:], in_=ps_y[:])
                nc.sync.dma_start(
                    y_d[mb * P:(mb + 1) * P, cc * 512:(cc + 1) * 512], ys[:]
                )

        def body(emit_consts):
            qkT = qk_pool.tile([P, 2 * HPC, T], BF16)    # q/k featT x tokens
            v_sb = v_pool.tile([P, NJB, HPC * D], BF16)  # tokens x v feats
            aT = a_pool.tile([P, HPC, T], BF16)          # attn outT per head
            for mc in range(MCH):
                if only in ("", "proj"):
                    proj_chunk(mc, qkT, v_sb, emit_consts and mc == 0)
                elif emit_consts and mc == 0:
                    load_consts()
                if only in ("", "attn"):
                    for h in range(HPC):
                        attn_chunk(h, mc, qkT, v_sb, aT)
                if only in ("", "outp"):
                    for mb in range(4 * mc, 4 * mc + 4):
                        outproj_chunk(mb, aT)
            if only == "proj":   # keep SBUF results live via a consumer
                nc.sync.dma_start(y_d[0:P, 0:256], qkT[:, 0, 0:512].bitcast(FP32))
                nc.sync.dma_start(y_d[P:2 * P, 0:256], v_sb[:, 0, :].bitcast(FP32))
            if only == "attn":
                nc.sync.dma_start(y_d[0:P, 0:256], aT[:, 0, 0:512].bitcast(FP32))

        if iters == 1:
            body(emit_consts=True)
        else:
            load_consts()
            with tc.For_i(0, iters, 1):
                body(emit_consts=False)
        if yext_d is not None:
            nc.sync.dma_start(yext_d[:], y_d[0:P, 0:512])

    nc.compile()
    return nc


def _rope_emb():
    freqs = np.arange(0, D, 2, dtype=np.float32) / D
    angles = np.arange(T, dtype=np.float32)[:, None] * freqs[None, :]
    return np.concatenate([np.sin(angles), np.cos(angles)], axis=-1)  # [T, D]


def make_in_maps(x, qkv_w, qkv_b, out_w):
    bf = ml_dtypes.bfloat16
    emb = _rope_emb()
    embq = np.ascontiguousarray((emb / math.sqrt(D)).T).astype(bf)  # [D, T]
    embk = np.ascontiguousarray(emb.T).astype(bf)
    # causal masks for the 4 diagonal-block offsets: m[jl, d*512+il]=1 iff il>=jl+128d
    jl = np.arange(P)[:, None]
    il = np.arange(512)[None, :]
    dmask = np.concatenate(
        [(il >= jl + 128 * d).astype(np.float32) for d in range(4)], axis=1
    ).astype(bf)
    in_maps = []
    for core in range(8):
        b, hg = divmod(core, 4)
        rq = slice(hg * 512, (hg + 1) * 512)
        rk = slice(C + hg * 512, C + (hg + 1) * 512)
        rv = slice(2 * C + hg * 512, 2 * C + (hg + 1) * 512)
        w_slice = np.concatenate([qkv_w[rq], qkv_w[rk], qkv_w[rv]], axis=0)
        wT = np.ascontiguousarray(w_slice.T).astype(bf)  # [C, 1536]
        bq = qkv_b[rq].reshape(HPC, P).T  # [128, 4] per-block columns
        bk = qkv_b[rk].reshape(HPC, P).T
        bqk = np.ascontiguousarray(np.concatenate([bq, bk], axis=1)).astype(np.float32)
        bv = np.ascontiguousarray(
            np.broadcast_to(qkv_b[rv][None, :], (P, HPC * D))
        ).astype(np.float32)
        woT = np.ascontiguousarray(out_w[:, hg * 512:(hg + 1) * 512].T).astype(bf)
        xT = np.ascontiguousarray(x[b].T).astype(bf)  # [C, T]
        in_maps.append({
            "xT": xT, "wT": wT, "bqk": bqk, "bv": bv,
            "embq": embq, "embk": embk, "woT": woT, "dmask": dmask,
        })
    return in_maps


_NC_CACHE = {}


def kernel(x, qkv_w, qkv_b, out_w, out_b):
    from concourse.bass_utils import run_bass_kernel_spmd

    x = np.asarray(x, dtype=np.float32)
    qkv_w = np.asarray(qkv_w, dtype=np.float32)
    qkv_b = np.asarray(qkv_b, dtype=np.float32)
    out_w = np.asarray(out_w, dtype=np.float32)
    out_b = np.asarray(out_b, dtype=np.float32)

    if "nc" not in _NC_CACHE:
        _NC_CACHE["nc"] = build_bass()
    nc = _NC_CACHE["nc"]

    in_maps = make_in_maps(x, qkv_w, qkv_b, out_w)
    res = run_bass_kernel_spmd(nc, in_maps, core_ids=list(range(8)))
    out = np.empty((B, T, C), dtype=np.float32)
    for b in range(B):
        acc = res.results[4 * b]["y"].astype(np.float32)
        for hg in range(1, 4):
            acc = acc + res.results[4 * b + hg]["y"]
        out[b] = acc + out_b[None, :]
    return out


# revision 35
# speedup vs baseline: 5.1499x; 1.1489x over previous
"""Multi-head causal attention (B=2, T=2048, C=2048, H=16, D=128) on 8 TRN2
NeuronCores.

Sharding: core = (batch, head_group): cores 0-3 -> batch 0, cores 4-7 ->
batch 1; head_group hg = core % 4 owns heads [4*hg, 4*hg+4).  Each core:
  qkv projection for its 4 heads (q/k transposed layout, v natural layout),
  multiplicative RoPE, causal attention with transposed scores, and the
  out-projection partial product over its 512 feature columns.
Host: per-batch sum of the 4 partial outputs + out bias (the "all-reduce").

Emission interleaves attention/out-proj chunks with projection m-chunks
(causality: query chunk ci only needs tokens < 512*(ci+1)) so the scalar/
vector-heavy softmax work overlaps the PE-heavy projection.
"""

import math
from contextlib import ExitStack

import ml_dtypes
import numpy as np

import concourse.bass as bass
import concourse.tile as tile
from concourse import bacc, bass_isa, mybir

BF16 = mybir.dt.bfloat16
FP32 = mybir.dt.float32
AF = mybir.ActivationFunctionType

B, T, C, H, D = 2, 2048, 2048, 16, 128
HPC = 4          # heads per core
P = 128
KO = C // P      # 16 k-tiles for the projection contraction
MCH = T // 512   # 4 m-chunks of 512 tokens
NJB = T // P     # 16 key blocks
NCI = T // 512   # 4 query chunks


def build_bass(iters: int = 1, timing_io: bool = False, no_pool: bool = False,
               only: str = ""):
    nc = bacc.Bacc("TRN2", target_bir_lowering=False, debug=False, num_devices=8)

    # timing_io: big tensors become Internal DRAM (contents garbage) so each
    # run ships only a tiny input/output over the axon tunnel
    kind = "Internal" if timing_io else "ExternalInput"

    def in_tensor(name, shape, dtype):
        if timing_io:
            return nc.dram_tensor(name, shape, dtype).ap()
        return nc.dram_tensor(name, shape, dtype, kind="ExternalInput").ap()

    xT_d = in_tensor("xT", [C, T], BF16)
    wT_d = in_tensor("wT", [C, 3 * HPC * D], BF16)
    bqk_d = nc.dram_tensor("bqk", [P, 2 * HPC], FP32, kind="ExternalInput").ap()
    bv_d = in_tensor("bv", [P, HPC * D], FP32)
    embq_d = in_tensor("embq", [P, T], BF16)
    embk_d = in_tensor("embk", [P, T], BF16)
    woT_d = in_tensor("woT", [HPC * D, C], BF16)
    dmask_d = in_tensor("dmask", [P, 4 * 512], BF16)
    if timing_io:
        y_d = nc.dram_tensor("y", [T, C], FP32).ap()
        yext_d = nc.dram_tensor("yext", [P, 512], FP32, kind="ExternalOutput").ap()
    else:
        y_d = nc.dram_tensor("y", [T, C], FP32, kind="ExternalOutput").ap()
        yext_d = None

    with tile.TileContext(nc) as tc, ExitStack() as ctx:
        cpool = ctx.enter_context(tc.tile_pool(name="consts", bufs=1))
        qk_pool = ctx.enter_context(tc.tile_pool(name="qk", bufs=1))
        v_pool = ctx.enter_context(tc.tile_pool(name="v", bufs=1))
        a_pool = ctx.enter_context(tc.tile_pool(name="aT", bufs=1))
        x_pool = ctx.enter_context(tc.tile_pool(name="xt", bufs=2))
        s_pool = ctx.enter_context(tc.tile_pool(name="es", bufs=8))
        r_pool = ctx.enter_context(tc.tile_pool(name="recip", bufs=3))
        rb_pool = ctx.enter_context(tc.tile_pool(name="recipb", bufs=3))
        y_pool = ctx.enter_context(tc.tile_pool(name="ys", bufs=6))
        ps_pool = ctx.enter_context(tc.tile_pool(name="ps", bufs=8, space="PSUM"))

        # const tiles (DMA emission deferred so xt chunk 0 leads the queue)
        w_sb = cpool.tile([P, KO, 3 * HPC * D], BF16)   # qkv weightsT
        woT_sb = cpool.tile([P, HPC, C], BF16)
        embq_sb = cpool.tile([P, T], BF16)
        embk_sb = cpool.tile([P, T], BF16)
        bqk_sb = cpool.tile([P, 2 * HPC], FP32)
        bv_sb = cpool.tile([P, HPC * D], FP32)
        ones_sb = cpool.tile([P, 1], BF16)
        ones_row = cpool.tile([1, P], FP32)
        dmask_sb = cpool.tile([P, 4 * 512], BF16)

        def load_consts():
            # smaller per-ko transfers so the first matmuls start early
            wTr = wT_d.rearrange("(ko p) n -> p ko n", p=P)
            for ko in range(KO):
                nc.sync.dma_start(w_sb[:, ko, :], wTr[:, ko, :])
            nc.sync.dma_start(bqk_sb[:], bqk_d)
            nc.sync.dma_start(bv_sb[:], bv_d)
            nc.sync.dma_start(embq_sb[:], embq_d)
            nc.sync.dma_start(embk_sb[:], embk_d)
            nc.sync.dma_start(woT_sb[:], woT_d.rearrange("(hh p) n -> p hh n", p=P))
            if no_pool:
                nc.sync.dma_start(dmask_sb[:], dmask_d)
            nc.vector.memset(ones_sb[:], 1.0)
            nc.vector.memset(ones_row[:], 1.0)

        xTr = xT_d.rearrange("(ko p) m -> p ko m", p=P)

        def proj_chunk(mc, qkT, v_sb, emit_consts):
            ms = slice(mc * 512, (mc + 1) * 512)
            xt = x_pool.tile([P, KO, 512], BF16, tag="xt", name=f"xt{mc}")
            nc.sync.dma_start(xt[:], xTr[:, :, ms])
            if emit_consts:
                load_consts()
            # three passes of 4 psum banks each: q, k, v
            for part in range(2):      # 0: q feats, 1: k feats
                ps = [ps_pool.tile([P, 512], FP32, tag="mm",
                                   name=f"psqk{mc}_{part}_{f}") for f in range(4)]
                for ko in range(KO):
                    for f in range(4):
                        fb = part * 4 + f
                        nc.tensor.matmul(
                            ps[f][:],
                            lhsT=w_sb[:, ko, fb * P:(fb + 1) * P],
                            rhs=xt[:, ko, :],
                            start=(ko == 0),
                            stop=(ko == KO - 1),
                        )
                for f in range(4):
                    fb = part * 4 + f
                    # fused bias + multiplicative RoPE: (psum + b) * emb on DVE
                    emb = embq_sb if part == 0 else embk_sb
                    nc.vector.scalar_tensor_tensor(
                        out=qkT[:, fb, ms], in0=ps[f][:],
                        scalar=bqk_sb[:, fb:fb + 1], in1=emb[:, ms],
                        op0=mybir.AluOpType.add, op1=mybir.AluOpType.mult,
                    )
            ps_v = [ps_pool.tile([P, 512], FP32, tag="mm",
                                 name=f"psv{mc}_{s}") for s in range(4)]
            for ko in range(KO):
                for s in range(4):
                    nc.tensor.matmul(
                        ps_v[s][:],
                        lhsT=xt[:, ko, s * P:(s + 1) * P],
                        rhs=w_sb[:, ko, 2 * HPC * D:3 * HPC * D],
                        start=(ko == 0),
                        stop=(ko == KO - 1),
                    )
            for s in range(4):
                nc.vector.tensor_add(
                    out=v_sb[:, mc * 4 + s, :], in0=ps_v[s][:], in1=bv_sb[:]
                )

        def attn_chunk(h, ci, qkT, v_sb, aT):
            iss = slice(ci * 512, (ci + 1) * 512)
            njb = 4 * (ci + 1)
            ps_o = ps_pool.tile([P, 512], FP32, tag="mm", name=f"pso{h}_{ci}")
            ps_d = ps_pool.tile([P, 512], FP32, tag="mm", name=f"psd{h}_{ci}")
            acc = r_pool.tile([P, 512], FP32, tag="acc", name=f"acc{h}_{ci}")
            for jb in range(njb):
                ps_s = ps_pool.tile([P, 512], FP32, tag="mm",
                                    name=f"pss{h}_{ci}_{jb}")
                nc.tensor.matmul(
                    ps_s[:],
                    lhsT=qkT[:, HPC + h, jb * P:(jb + 1) * P],
                    rhs=qkT[:, h, iss],
                    start=True, stop=True,
                )
                es = s_pool.tile([P, 512], BF16, tag="es", name=f"es{h}_{ci}_{jb}")
                diag = jb >= 4 * ci
                delta = jb * P - ci * 512
                if no_pool:
                    nc.scalar.activation(out=es[:], in_=ps_s[:], func=AF.Exp)
                    if diag:  # multiplicative causal mask, const tile on DVE
                        di = delta // P
                        nc.vector.tensor_mul(
                            out=es[:], in0=es[:],
                            in1=dmask_sb[:, di * 512:(di + 1) * 512],
                        )
                elif diag:
                    # columns < delta are fully masked: skip their exp
                    if delta > 0:
                        nc.gpsimd.memset(es[:, :delta], 0.0)
                    nc.scalar.activation(
                        out=es[:, delta:], in_=ps_s[:, delta:], func=AF.Exp
                    )
                    nc.gpsimd.affine_select(
                        out=es[:, delta:], in_=es[:, delta:],
                        compare_op=mybir.AluOpType.is_ge,
                        fill=0.0, base=0,
                        pattern=[[1, 512 - delta]], channel_multiplier=-1,
                    )
                else:
                    nc.scalar.activation(out=es[:], in_=ps_s[:], func=AF.Exp)
                nc.tensor.matmul(
                    ps_o[:],
                    lhsT=v_sb[:, jb, h * D:(h + 1) * D],
                    rhs=es[:],
                    start=(jb == 0), stop=(jb == njb - 1),
                )
                if no_pool:
                    # denominator entirely on PE
                    nc.tensor.matmul(
                        ps_d[0:1, :],
                        lhsT=ones_sb[:],
                        rhs=es[:],
                        start=(jb == 0), stop=(jb == njb - 1),
                    )
                elif jb == 0:
                    nc.vector.tensor_copy(out=acc[:], in_=es[:])
                elif jb % 2 == 0:
                    nc.vector.tensor_add(out=acc[:], in0=acc[:], in1=es[:])
                else:
                    nc.tensor.matmul(
                        ps_d[0:1, :],
                        lhsT=ones_sb[:],
                        rhs=es[:],
                        start=(jb == 1), stop=(jb == njb - 1),
                    )
            rb = rb_pool.tile([P, 512], FP32, tag="rb", name=f"rb{h}_{ci}")
            if no_pool:
                rr = r_pool.tile([1, 512], FP32, tag="acc", name=f"rr{h}_{ci}")
                nc.vector.reciprocal(out=rr[:], in_=ps_d[0:1, :])
                # broadcast across partitions as a rank-1 outer product on PE
                ps_rb = ps_pool.tile([P, 512], FP32, tag="mm", name=f"psrb{h}_{ci}")
                nc.tensor.matmul(
                    ps_rb[:], lhsT=ones_row[:], rhs=rr[:], start=True, stop=True
                )
                nc.vector.tensor_copy(out=rb[:], in_=ps_rb[:])
            else:
                nc.vector.tensor_add(
                    out=acc[0:1, :], in0=acc[0:1, :], in1=ps_d[0:1, :]
                )
                nc.gpsimd.partition_all_reduce(
                    rb[:], acc[:], channels=P, reduce_op=bass_isa.ReduceOp.add
                )
                nc.vector.reciprocal(out=rb[:], in_=rb[:])
            nc.vector.tensor_mul(out=aT[:, h, iss], in0=ps_o[:], in1=rb[:])

        def outproj_chunk(mb, aT):
            for cc in range(C // 512):
                ps_y = ps_pool.tile([P, 512], FP32, tag="mm", name=f"psy{mb}_{cc}")
                for h in range(HPC):
                    nc.tensor.matmul(
                        ps_y[:],
                        lhsT=aT[:, h, mb * P:(mb + 1) * P],
                        rhs=woT_sb[:, h, cc * 512:(cc + 1) * 512],
                        start=(h == 0), stop=(h == HPC - 1),
                    )
                ys = y_pool.tile([P, 512], FP32, tag="ys", name=f"ys{mb}_{cc}")
                # alternate drain engine to split the load
                if (mb + cc) % 2 == 0:
                    nc.vector.tensor_copy(out=ys[:], in_=ps_y[:])
                else:
                    nc.scalar.copy(out=ys[:], in_=ps_y[:])
                nc.sync.dma_start(
                    y_d[mb * P:(mb + 1) * P, cc * 512:(cc + 1) * 512], ys[:]
                )

        def body(emit_consts):
            qkT = qk_pool.tile([P, 2 * HPC, T], BF16)    # q/k featT x tokens
            v_sb = v_pool.tile([P, NJB, HPC * D], BF16)  # tokens x v feats
            aT = a_pool.tile([P, HPC, T], BF16)          # attn outT per head
            for mc in range(MCH):
                if only in ("", "proj"):
                    proj_chunk(mc, qkT, v_sb, emit_consts and mc == 0)
                elif emit_consts and mc == 0:
                    load_consts()
                if only in ("", "attn"):
                    for h in range(HPC):
                        attn_chunk(h, mc, qkT, v_sb, aT)
                if only in ("", "outp"):
                    for mb in range(4 * mc, 4 * mc + 4):
                        outproj_chunk(mb, aT)
            if only == "proj":   # keep SBUF results live via a consumer
                nc.sync.dma_start(y_d[0:P, 0:256], qkT[:, 0, 0:512].bitcast(FP32))
                nc.sync.dma_start(y_d[P:2 * P, 0:256], v_sb[:, 0, :].bitcast(FP32))
            if only == "attn":
                nc.sync.dma_start(y_d[0:P, 0:256], aT[:, 0, 0:512].bitcast(FP32))

        if iters == 1:
            body(emit_consts=True)
        else:
            load_consts()
            with tc.For_i(0, iters, 1):
                body(emit_consts=False)
        if yext_d is not None:
            nc.sync.dma_start(yext_d[:], y_d[0:P, 0:512])

    nc.compile()
    return nc


def _rope_emb():
    freqs = np.arange(0, D, 2, dtype=np.float32) / D
    angles = np.arange(T, dtype=np.float32)[:, None] * freqs[None, :]
    return np.concatenate([np.sin(angles), np.cos(angles)], axis=-1)  # [T, D]


def make_in_maps(x, qkv_w, qkv_b, out_w):
    bf = ml_dtypes.bfloat16
    emb = _rope_emb()
    embq = np.ascontiguousarray((emb / math.sqrt(D)).T).astype(bf)  # [D, T]
    embk = np.ascontiguousarray(emb.T).astype(bf)
    # causal masks for the 4 diagonal-block offsets: m[jl, d*512+il]=1 iff il>=jl+128d
    jl = np.arange(P)[:, None]
    il = np.arange(512)[None, :]
    dmask = np.concatenate(
        [(il >= jl + 128 * d).astype(np.float32) for d in range(4)], axis=1
    ).astype(bf)
    in_maps = []
    for core in range(8):
        b, hg = divmod(core, 4)
        rq = slice(hg * 512, (hg + 1) * 512)
        rk = slice(C + hg * 512, C + (hg + 1) * 512)
        rv = slice(2 * C + hg * 512, 2 * C + (hg + 1) * 512)
        w_slice = np.concatenate([qkv_w[rq], qkv_w[rk], qkv_w[rv]], axis=0)
        wT = np.ascontiguousarray(w_slice.T).astype(bf)  # [C, 1536]
        bq = qkv_b[rq].reshape(HPC, P).T  # [128, 4] per-block columns
        bk = qkv_b[rk].reshape(HPC, P).T
        bqk = np.ascontiguousarray(np.concatenate([bq, bk], axis=1)).astype(np.float32)
        bv = np.ascontiguousarray(
            np.broadcast_to(qkv_b[rv][None, :], (P, HPC * D))
        ).astype(np.float32)
        woT = np.ascontiguousarray(out_w[:, hg * 512:(hg + 1) * 512].T).astype(bf)
        xT = np.ascontiguousarray(x[b].T).astype(bf)  # [C, T]
        in_maps.append({
            "xT": xT, "wT": wT, "bqk": bqk, "bv": bv,
            "embq": embq, "embk": embk, "woT": woT, "dmask": dmask,
        })
    return in_maps


_NC_CACHE = {}


def kernel(x, qkv_w, qkv_b, out_w, out_b):
    from concourse.bass_utils import run_bass_kernel_spmd

    x = np.asarray(x, dtype=np.float32)
    qkv_w = np.asarray(qkv_w, dtype=np.float32)
    qkv_b = np.asarray(qkv_b, dtype=np.float32)
    out_w = np.asarray(out_w, dtype=np.float32)
    out_b = np.asarray(out_b, dtype=np.float32)

    if "nc" not in _NC_CACHE:
        _NC_CACHE["nc"] = build_bass()
    nc = _NC_CACHE["nc"]

    in_maps = make_in_maps(x, qkv_w, qkv_b, out_w)
    res = run_bass_kernel_spmd(nc, in_maps, core_ids=list(range(8)))
    out = np.empty((B, T, C), dtype=np.float32)
    for b in range(B):
        acc = res.results[4 * b]["y"].astype(np.float32)
        for hg in range(1, 4):
            acc = acc + res.results[4 * b + hg]["y"]
        out[b] = acc + out_b[None, :]
    return out
